# revision 31
# baseline (speedup 1.0000x reference)
"""Trainium2 Bass kernel for DCRNN-Temporal (gnn_message_passing).

Contract: kernel(**inputs) takes FULL numpy inputs (x, edge_index,
edge_weight, w_z, b_z, w_r, b_r, w_h, b_h, w_lin, b_lin) and returns the
FULL [N, 12] output, running a Bass SPMD kernel on 8 NeuronCores.

Math (H0 = 0 simplifies the DCRNN cell):
  R is unused (H0*R = 0), both remaining DConvs share the same diffusion
  features. With per-edge weights 1/deg(src) folded into pre-scaled
  tables and the Chebyshev recurrence folded into effective weights:
    T1o = P_f x, T1i = P_r x, Y2o = P_f T1o, Y2i = P_r T1i
    G   = [x | T1o, T1i, Y2o, Y2i] @ Weff + beff    (z | h gates)
    H   = sigmoid(-Gz - bz) * tanh(Gh + bh)         (= (1-Z)*Htilde)
    out = relu(H) @ w_lin + b_lin

Sharding: nodes partitioned by destination across 8 cores.  Each
propagate = dma_gather row gathers (256B rows) + DVE adds over
degree-sorted ELL rounds (scatter-free).  dma_gather indices are int16,
so every node table uses one global 6256-periodic layout (6250 node
rows + 6 zero rows per core; gid(v) = (v//6250)*6256 + v%6250 < 50048)
and each direction's edges split into two windowed streams:
  A: gid < 32768 (idx = gid),  B: idx = gid - 32768 (table AP offset).

Performance structure (v3): SWDGE descriptor generation is the
bottleneck (~7.5ns/idx per queue, 4 queues generating concurrently).
The 8 gather streams (2 hops x 2 dirs x 2 windows) issue back-to-back;
each stream's un-permute scatter_adds are issued in REVERSE column
order (low-degree accumulator columns receive their last ELL add early)
and injected into the NEXT stream's gather rotation, so the gpsimd
in-order queue never stalls on yt-finalization.  Merge reloads ride the
scalar queue; ix index loads own the sync queue.  The AllGathers are
injected mid-stream so their transfer hides under SWDGE work.  The gate
computation is split: x/T1 parts during hop-2 fwd, Y2o part during
hop-2 rev, and only the Y2i part in the tail, pipelined per 896-node
chunk behind the final scatters.
"""

import os
import sys

for _p in ("/opt/trn_rl_repo", "/root/.axon_site/_ro/trn_rl_repo"):
    if os.path.isdir(_p) and _p not in sys.path:
        sys.path.insert(0, _p)
        break

import numpy as np

import concourse.bass as bass
import concourse.tile as tile
from concourse import bacc, mybir
from concourse import bass_utils
from concourse.masks import make_identity

F = 32          # node feature dim
FO = 64         # out channels per gate
GATES = 2 * FO
PER = 12        # head output dim
NCORES = 8
ES = 64         # table row length (f32) = 256B, required by dma_gather
CH = 7          # SWDGE chunk width in 128-cols (<=896 descriptors/call)
NT = 512        # node tile (free dim) for gate matmuls
WINDOW = 32768  # int16 idx window
GAPZ = 6        # zero rows appended per core in the global table layout
NQ = 4          # SWDGE queues (ucode max); round-robin for parallel gen

FP32 = mybir.dt.float32
I16 = mybir.dt.int16


def _ceil(a, b):
    return -(-a // b)


def _wrap16(idx_seq):
    """[n] -> [128, n//16] int16, wrapped in 16 partitions, replicated 8x."""
    w = idx_seq.reshape(-1, 16).T.astype(np.int16)
    return np.ascontiguousarray(np.tile(w, (8, 1)))


# ----------------------------------------------------------------------------
# Host-side graph preprocessing
# ----------------------------------------------------------------------------

def _build_streams(dst, src_gid, N, NLOC, P, W):
    """ELL-round structures for one directed edge set, split into
    idx-window streams A (gid < W) and B (gid >= W)."""
    core = dst // NLOC
    dl = dst - core * NLOC
    streams = []
    for s, mask in (("A", src_gid < W), ("B", src_gid >= W)):
        percore = []
        for p in range(P):
            sel = np.nonzero((core == p) & mask)[0]
            cnt = np.bincount(dl[sel], minlength=NLOC)
            perm = np.argsort(-cnt, kind="stable").astype(np.int32)
            rank = np.empty(NLOC, np.int32)
            rank[perm] = np.arange(NLOC, dtype=np.int32)
            order = np.argsort(dl[sel], kind="stable")
            es = sel[order]
            ptr = np.zeros(NLOC + 1, np.int64)
            np.cumsum(cnt, out=ptr[1:])
            j_arr = np.arange(len(es), dtype=np.int64) - ptr[dl[es]]
            percore.append(dict(es=es, j=j_arr, r=rank[dl[es]].astype(np.int64),
                                cnt=cnt, perm=perm))
        maxdeg = int(max(pc["cnt"].max(initial=0) for pc in percore))
        NJ = []
        for j in range(maxdeg):
            nj = max(int((pc["cnt"] > j).sum()) for pc in percore)
            NJ.append(_ceil(nj, 128) * 128)
        off = np.zeros(len(NJ) + 1, np.int64)
        np.cumsum(NJ, out=off[1:])
        EP = int(off[-1])
        for pc in percore:
            pc["slot"] = off[pc["j"]] + pc["r"]
        streams.append(dict(name=s, percore=percore, NJ=NJ, off=off, EP=EP))
    return streams


def preprocess(x, edge_index, edge_weight, w_z, b_z, w_r, b_r, w_h, b_h,
               w_lin, b_lin, P=NCORES, window=WINDOW):
    N, Fx = x.shape
    assert Fx == F
    assert N % P == 0
    NLOC = N // P
    NLOCP = _ceil(NLOC, 128) * 128
    CACC = NLOCP // 128
    NL6 = NLOC + GAPZ
    NTAB = P * NL6
    W = min(window, NTAB)
    assert NTAB - W < 32768 and NLOC < W

    row = np.asarray(edge_index[0], dtype=np.int64)
    col = np.asarray(edge_index[1], dtype=np.int64)
    ew = np.asarray(edge_weight, dtype=np.float64)
    deg_out = np.bincount(row, weights=ew, minlength=N)
    deg_in = np.bincount(col, weights=ew, minlength=N)
    with np.errstate(divide="ignore"):
        doi = np.where(deg_out > 0, 1.0 / deg_out, 0.0).astype(np.float32)
        dii = np.where(deg_in > 0, 1.0 / deg_in, 0.0).astype(np.float32)

    gid = (np.arange(N) // NLOC) * NL6 + (np.arange(N) % NLOC)
    xf = np.asarray(x, np.float32)

    def table(scaled):
        t = np.zeros((NTAB, ES), np.float32)
        t[gid, :F] = scaled
        return t

    XF = table(xf * doi[:, None])
    XR = table(xf * dii[:, None])

    # pad slot targets: a zero row inside each window
    apad = NLOC  # core-0 zero row, < W
    zq = _ceil(W - NLOC, NL6)  # first core whose zero row is >= W
    bpad = zq * NL6 + NLOC - W
    assert 0 <= bpad < NTAB - W or W == NTAB

    gsrcF = gid[row]
    gsrcR = gid[col]
    stF = _build_streams(col, gsrcF, N, NLOC, P, W)
    stR = _build_streams(row, gsrcR, N, NLOC, P, W)

    # effective gate weights (K = 3); WA rows = [T1o, T1i, Y2o, Y2i] so the
    # T1 half [0:2F] can matmul early (during hop 2) with contiguous lhsT
    assert w_z.shape[1] == 3

    def gate_w(w):
        w = np.asarray(w, np.float32)
        wx = w[0, 0, :F] + w[1, 0, :F] - w[0, 2, :F] - w[1, 2, :F]
        wa = np.concatenate(
            [w[0, 1, :F], w[1, 1, :F], 2.0 * w[0, 2, :F], 2.0 * w[1, 2, :F]], 0)
        return wx, wa

    wxz, waz = gate_w(w_z)
    wxh, wah = gate_w(w_h)
    WA = np.ascontiguousarray(np.concatenate([waz, wah], axis=1))
    WX = np.ascontiguousarray(np.concatenate([wxz, wxh], axis=1))
    biasS = np.ascontiguousarray(-np.asarray(b_z, np.float32)[:, None])
    biasT = np.ascontiguousarray(np.asarray(b_h, np.float32)[:, None])
    WL = np.asarray(w_lin, np.float32)
    BL = np.ascontiguousarray(np.asarray(b_lin, np.float32)[:, None])

    def expand(vec):  # [NLOCP] -> [128, CACC, F] accumulator-layout expand
        return np.ascontiguousarray(
            np.repeat(vec.reshape(CACC, 128).T, F, axis=1)
        ).reshape(128, CACC, F)

    in_maps = []
    for p in range(P):
        m = {"xf": XF, "xr": XR, "wa": WA, "wx": WX, "biass": biasS,
             "biast": biasT, "wl": WL, "bl": BL}
        for dname, st, gsrc, srcv in (("f", stF, gsrcF, row),
                                      ("r", stR, gsrcR, col)):
            for s in st:
                pc = s["percore"][p]
                if s["EP"] == 0:
                    continue
                base, pad = (0, apad) if s["name"] == "A" else (W, bpad)
                ivals = np.full(s["EP"], pad, np.int64)
                ivals[pc["slot"]] = gsrc[pc["es"]] - base
                m[f"ix{dname}{s['name'].lower()}"] = _wrap16(ivals)
                sidx = np.full(NLOCP, NLOC, np.int64)
                sidx[:NLOC] = pc["perm"]
                m[f"sx{dname}{s['name'].lower()}"] = _wrap16(sidx)
        sl = slice(p * NLOC, (p + 1) * NLOC)
        xp = np.zeros((NLOCP, F), np.float32)
        xp[:NLOC] = xf[sl]
        m["xpt"] = np.ascontiguousarray(xp.T)
        tmp = np.zeros(NLOCP, np.float32)
        tmp[:NLOC] = doi[sl]
        m["doe"] = expand(tmp)
        tmp = np.zeros(NLOCP, np.float32)
        tmp[:NLOC] = dii[sl]
        m["die"] = expand(tmp)
        in_maps.append(m)

    meta = dict(N=N, P=P, NLOC=NLOC, NLOCP=NLOCP, CACC=CACC, NL6=NL6,
                NTAB=NTAB, W=W,
                streams={"f": [dict(name=s["name"], NJ=s["NJ"], off=s["off"],
                                    EP=s["EP"]) for s in stF],
                         "r": [dict(name=s["name"], NJ=s["NJ"], off=s["off"],
                                    EP=s["EP"]) for s in stR]})
    return in_maps, meta


# ----------------------------------------------------------------------------
# Device program
# ----------------------------------------------------------------------------

def build_program(meta, debug=False):
    from contextlib import ExitStack

    N, P = meta["N"], meta["P"]
    NLOC, NLOCP, CACC = meta["NLOC"], meta["NLOCP"], meta["CACC"]
    NL6, NTAB, W = meta["NL6"], meta["NTAB"], meta["W"]
    streams = meta["streams"]
    NSC = _ceil(CACC, CH)  # scatter chunks per stream (7)

    nc = bacc.Bacc("TRN2", target_bir_lowering=False, debug=False,
                   num_devices=P, num_swdge_queues=NQ)

    def din(name, shape, dt=FP32):
        return nc.dram_tensor(name, list(shape), dt, kind="ExternalInput").ap()

    def dout(name, shape, dt=FP32):
        return nc.dram_tensor(name, list(shape), dt, kind="ExternalOutput").ap()

    xf_d = din("xf", (NTAB, ES))
    xr_d = din("xr", (NTAB, ES))
    ix_d, sx_d = {}, {}
    for d in ("f", "r"):
        for s in streams[d]:
            if s["EP"] == 0:
                continue
            k = d + s["name"].lower()
            ix_d[k] = din("ix" + k, (128, s["EP"] // 16), I16)
            sx_d[k] = din("sx" + k, (128, NLOCP // 16), I16)
    xpt_d = din("xpt", (F, NLOCP))
    doe_d = din("doe", (128, CACC, F))
    die_d = din("die", (128, CACC, F))
    wa_d = din("wa", (4 * F, GATES))
    wx_d = din("wx", (F, GATES))
    biass_d = din("biass", (FO, 1))
    biast_d = din("biast", (FO, 1))
    wl_d = din("wl", (FO, PER))
    bl_d = din("bl", (PER, 1))
    out_d = dout("out", (PER, NLOCP))

    EPmax = max(s["EP"] for d in ("f", "r") for s in streams[d])
    qctr = [0]

    def nextq():
        q = qctr[0] % NQ
        qctr[0] += 1
        return q

    with tile.TileContext(nc) as tc, ExitStack() as ctx:
        sb = ctx.enter_context(tc.tile_pool(name="sb", bufs=1))
        ya = ctx.enter_context(tc.tile_pool(name="ya", bufs=3))
        gp = ctx.enter_context(tc.tile_pool(name="gp", bufs=9))
        ixa = ctx.enter_context(tc.tile_pool(name="ixa", bufs=1))
        ixb = ctx.enter_context(tc.tile_pool(name="ixb", bufs=1))
        big = ctx.enter_context(tc.tile_pool(name="big", bufs=1))
        mg = ctx.enter_context(tc.tile_pool(name="mg", bufs=1))
        tl = ctx.enter_context(tc.tile_pool(name="tl", bufs=4))
        px = ctx.enter_context(tc.tile_pool(name="px", bufs=2))
        pp = ctx.enter_context(tc.tile_pool(name="pp", bufs=2, space="PSUM"))
        pt = ctx.enter_context(tc.tile_pool(name="pt", bufs=2, space="PSUM"))
        ph = ctx.enter_context(tc.tile_pool(name="ph", bufs=2, space="PSUM"))
        dr = ctx.enter_context(tc.tile_pool(name="dr", bufs=1, space="DRAM"))

        phi = sb.tile([128, NLOCP], FP32, tag="phi")
        gPZ = sb.tile([FO, NLOCP], FP32, tag="gPZ")
        gPH = sb.tile([FO, NLOCP], FP32, tag="gPH")
        doe = sb.tile([128, CACC, F], FP32, tag="doe")
        die = sb.tile([128, CACC, F], FP32, tag="die")
        sxt = {}
        for d in ("f", "r"):
            for s in streams[d]:
                if s["EP"] == 0:
                    continue
                k = d + s["name"].lower()
                sxt[k] = sb.tile([128, NLOCP // 16], I16, tag="sx" + k,
                                 name="sx" + k)
        wa = sb.tile([4 * F, GATES], FP32, tag="wa")
        wa3 = sb.tile([F, GATES], FP32, tag="wa3")  # Y2i block at base 0
        wx = sb.tile([F, GATES], FP32, tag="wx")
        bS = sb.tile([FO, 1], FP32, tag="bS")
        bT = sb.tile([FO, 1], FP32, tag="bT")
        wl = sb.tile([FO, PER], FP32, tag="wl")
        bl = sb.tile([PER, 1], FP32, tag="bl")
        ident = sb.tile([128, 128], FP32, tag="ident")

        # yall holds natural-order merged T1 blocks (T1o | T1i); Y2o gets its
        # own tile; the Y2i block goes straight to phi in the tail.
        yall = big.tile([128, CACC, 2 * F], FP32, tag="big")
        y2o = sb.tile([128, CACC, F], FP32, tag="y2o")

        NBNC = _ceil(max(NLOCP, NL6), 128) * 128
        # per-stream scatter targets (A/B separated so their scatter_adds
        # run concurrently on different queues; merged on reload)
        bnd = {}
        for hop_i in (1, 2):
            for d in ("f", "r"):
                for s in ("a", "b"):
                    nm = f"bnd{hop_i}{d}{s}"
                    bnd[(hop_i, d, s)] = dr.tile([NBNC, ES], FP32,
                                                 tag=nm, name=nm)
        bounS = {d: dr.tile([NBNC, ES], FP32, tag="bounS" + d,
                            name="bounS" + d) for d in ("f", "r")}
        # Shared HBM output lets the AllGather write in place (each core
        # deposits its slab) instead of the slow HBM-HBM collective path.
        ag = {d: nc.dram_tensor("agsh" + d, [NTAB, ES], FP32,
                                addr_space="Shared").ap()
              for d in ("f", "r")}

        def wrap(dram_tile, c0, cn):
            apv = dram_tile[:].rearrange("(c p) f -> p c f", p=128)
            return apv[:, :, c0:c0 + cn]

        # --- loads: ix of the first stream rides the sync queue alone;
        # everything else goes on scalar so the first gather starts asap.
        for k, t in sxt.items():
            nc.scalar.dma_start(t[:], sx_d[k])
        nc.scalar.dma_start(doe[:], doe_d)
        nc.scalar.dma_start(die[:], die_d)
        nc.scalar.dma_start(wa[:], wa_d)
        nc.scalar.dma_start(wa3[:], wa_d[3 * F:4 * F, :])
        nc.scalar.dma_start(wx[:], wx_d)
        nc.scalar.dma_start(bS[:], biass_d)
        nc.scalar.dma_start(bT[:], biast_d)
        nc.scalar.dma_start(wl[:], wl_d)
        nc.scalar.dma_start(bl[:], bl_d)
        make_identity(nc, ident[:])

        # --- prezero DRAM scatter targets (scalar queue, early) ------------
        z0 = ya.tile([128, CACC, ES], FP32, tag="y", name="z0")
        nc.vector.memset(z0[:], 0.0)
        for t in bnd.values():
            nc.scalar.dma_start(wrap(t, 0, ES)[:, :CACC, :], z0[:])

        # --- stream machinery ----------------------------------------------
        def prop_stream(yt, sdesc, ixkey, tabA, tabB, estep, inject):
            """Gather + ELL-round accumulate for one windowed stream.
            `inject` maps chunk index -> list of closures to emit there
            (scatters of previous streams, merge reloads, AllGathers)."""
            EP = sdesc["EP"]
            EPc = EP // 128
            roff = [int(o) // 128 for o in sdesc["off"]]
            nrounds = len(sdesc["NJ"])
            table_ap = tabA if sdesc["name"] == "A" else tabB
            pool = ixa if sdesc["name"] == "A" else ixb
            ixt = pool.tile([128, EPmax // 16 if sdesc["name"] == "A"
                             else EP // 16], I16, tag="ix")
            head = min(4 * CH * 8, EP // 16)
            nc.sync.dma_start(ixt[:, :head], ix_d[ixkey][:, :head])
            if head < EP // 16:
                nc.sync.dma_start(ixt[:, head:EP // 16],
                                  ix_d[ixkey][:, head:])
            nch = _ceil(EPc, CH)
            for ci, c0 in enumerate(range(0, EPc, CH)):
                for fn in inject.pop(ci, ()):
                    fn()
                c1 = min(c0 + CH, EPc)
                g = gp.tile([128, CH, ES], FP32, tag="g")
                nc.gpsimd.dma_gather(
                    out_ap=g[:, :c1 - c0, :],
                    in_ap=table_ap,
                    idxs_ap=ixt[:, c0 * 8:c1 * 8],
                    num_idxs=(c1 - c0) * 128,
                    num_idxs_reg=(c1 - c0) * 128,
                    elem_size=ES,
                    elem_step=estep,
                    queue_num=nextq(),
                )
                for j in range(nrounds):
                    s = max(roff[j], c0)
                    e = min(roff[j + 1], c1)
                    if s >= e:
                        continue
                    ys = s - roff[j]
                    nc.vector.tensor_tensor(
                        out=yt[:, ys:ys + e - s, 0:F],
                        in0=yt[:, ys:ys + e - s, 0:F],
                        in1=g[:, s - c0:e - c0, 0:F],
                        op=mybir.AluOpType.add,
                    )
            # flush any injections past the end
            for ci in sorted(inject):
                for fn in inject.pop(ci):
                    fn()

        def mk_scatter(yt, key, tgt, k):
            """Scatter chunk k (columns [7k, 7k+7)) of yt into tgt."""
            def fn():
                c0 = k * CH
                c1 = min(c0 + CH, CACC)
                nc.gpsimd.dma_scatter_add(
                    out_ap=tgt[:],
                    in_ap=yt[:, c0:c1, :],
                    idxs_ap=sxt[key][:, c0 * 8:c1 * 8],
                    num_idxs=(c1 - c0) * 128,
                    num_idxs_reg=(c1 - c0) * 128,
                    elem_size=ES,
                    queue_num=nextq(),
                )
            return fn

        def add_inject(inject, closures, cis):
            for fn, ci in zip(closures, cis):
                inject.setdefault(ci, []).append(fn)

        def mk_merge(hop_i, d, blk, scale_t):
            """Reload A+B scatter buffers (scalar queue), merge into yall
            block `blk` (or y2o); for hop 1 also write the scaled AllGather
            input.  Returns (reload_closure, combine_closure)."""
            dst = (y2o[:, :, :] if blk == 2
                   else yall[:, :, blk * F:(blk + 1) * F])
            tmp = mg.tile([128, CACC, F], FP32, tag="mgl", name=f"mg{hop_i}{d}")

            def reload():
                nc.sync.dma_start(
                    dst, wrap(bnd[(hop_i, d, "a")], 0, F)[:, :CACC, :])
                nc.sync.dma_start(
                    tmp[:], wrap(bnd[(hop_i, d, "b")], 0, F)[:, :CACC, :])

            def combine():
                nc.vector.tensor_tensor(out=dst, in0=dst, in1=tmp[:],
                                        op=mybir.AluOpType.add)
                if scale_t is None:
                    return
                ts = mg.tile([128, CACC, F], FP32, tag="mgs")
                nc.vector.tensor_tensor(out=ts[:], in0=dst, in1=scale_t[:],
                                        op=mybir.AluOpType.mult)
                nc.sync.dma_start(wrap(bounS[d], 0, F)[:, :CACC, :], ts[:])

            return reload, combine

        def mk_ag(d):
            def fn():
                nc.gpsimd.collective_compute(
                    "AllGather", mybir.AluOpType.bypass,
                    replica_groups=[list(range(P))],
                    ins=[bounS[d][0:NL6, :].opt()],
                    outs=[ag[d][0:NTAB, :].opt()],
                )
            return fn

        # --- emission ------------------------------------------------------
        sdesc = {}
        for d in ("f", "r"):
            for s in streams[d]:
                sdesc[d + s["name"].lower()] = s

        tabs = {
            1: {"f": (xf_d[0:W, :], xf_d[W:NTAB, :]),
                "r": (xr_d[0:W, :], xr_d[W:NTAB, :])},
            2: {"f": (ag["f"][0:W, :], ag["f"][W:NTAB, :]),
                "r": (ag["r"][0:W, :], ag["r"][W:NTAB, :])},
        }

        LATE_SPREAD = [1, 3, 5, 7, 9, 11, 13]

        def scatter_points(s):
            """Safe self-injection chunk index for each scatter chunk k:
            columns [7k, 7k+7) receive their last ELL add once the final
            round with NJ > 896k has been gathered (+2 chunks of drain
            margin).  Chunks whose point falls past the stream end are
            returned separately for injection into the next stream."""
            EPc = s["EP"] // 128
            nch = _ceil(EPc, CH)
            inj, late = [], []
            for k in reversed(range(NSC)):
                lim = 128 * CH * k
                jlast = max(j for j, nj in enumerate(s["NJ"]) if nj > lim)
                ci = _ceil(int(s["off"][jlast + 1]), 128 * CH) + 2
                (inj if ci < nch else late).append((k, ci))
            return inj, late

        order = [(1, "f"), (1, "r"), (2, "f"), (2, "r")]
        yts = {}
        pending = []          # leftover scatter closures for the next stream
        pending_merge = []    # (reload, combine, agfn or None)

        AF = mybir.ActivationFunctionType

        def early_phase():
            # T1 transposes (2 cols per 128x128 PE transpose) + x/T1 partial
            # gates (runs during hop 2 fwd); psum->SBUF copies ride the
            # scalar engine so the DVE queue stays clear for ELL adds
            for t0 in range(0, CACC, 2):
                tn = min(2, CACC - t0)
                ps = pt.tile([128, 128], FP32, tag="ps")
                nc.tensor.transpose(
                    out=ps[0:tn * 2 * F, :],
                    in_=yall[:, t0:t0 + tn, :].rearrange("p a b -> p (a b)"),
                    identity=ident[:])
                for k in range(tn):
                    t = t0 + k
                    nc.scalar.copy(phi[0:FO, t * 128:(t + 1) * 128],
                                   ps[k * 2 * F:(k + 1) * 2 * F, :])
            for n0 in range(0, NLOCP, NT):
                n1 = min(n0 + NT, NLOCP)
                w = n1 - n0
                pxt = px.tile([F, NT], FP32, tag="pxt")
                nc.sync.dma_start(pxt[:, :w], xpt_d[:, n0:n1])
                pg = pp.tile([GATES, NT], FP32, tag="pg")
                nc.tensor.matmul(out=pg[:, :w], lhsT=wa[0:2 * F, :],
                                 rhs=phi[0:FO, n0:n1], start=True, stop=False)
                nc.tensor.matmul(out=pg[:, :w], lhsT=wx[:, :],
                                 rhs=pxt[:, :w], start=False, stop=True)
                nc.scalar.copy(gPZ[:, n0:n1], pg[0:FO, :w])
                nc.scalar.copy(gPH[:, n0:n1], pg[FO:GATES, :w])

        def blk2_phase():
            # Y2o transposes + partial gates accumulated into gPZ/gPH
            # (runs during hop 2 rev)
            for t0 in range(0, CACC, 4):
                tn = min(4, CACC - t0)
                ps = pt.tile([128, 128], FP32, tag="ps")
                nc.tensor.transpose(
                    out=ps[0:tn * F, :],
                    in_=y2o[:, t0:t0 + tn, :].rearrange("p a b -> p (a b)"),
                    identity=ident[:])
                for k in range(tn):
                    t = t0 + k
                    nc.scalar.copy(phi[FO:FO + F, t * 128:(t + 1) * 128],
                                   ps[k * F:(k + 1) * F, :])
            for n0 in range(0, NLOCP, NT):
                n1 = min(n0 + NT, NLOCP)
                w = n1 - n0
                pg = pp.tile([GATES, NT], FP32, tag="pg")
                nc.tensor.matmul(out=pg[:, :w], lhsT=wa[2 * F:3 * F, :],
                                 rhs=phi[FO:FO + F, n0:n1],
                                 start=True, stop=True)
                nc.vector.tensor_tensor(out=gPZ[:, n0:n1], in0=gPZ[:, n0:n1],
                                        in1=pg[0:FO, :w], op=mybir.AluOpType.add)
                nc.vector.tensor_tensor(out=gPH[:, n0:n1], in0=gPH[:, n0:n1],
                                        in1=pg[FO:GATES, :w],
                                        op=mybir.AluOpType.add)

        for si, (hop_i, d) in enumerate(order):
            for sname in ("a", "b"):
                key = d + sname
                s = sdesc[key]
                inject = {}
                # leftover scatter chunks of the previous stream
                if pending:
                    add_inject(inject, pending, LATE_SPREAD)
                    pending = []
                # merge + AllGather of the previous hop (A-streams): the
                # previous streams fully self-scattered, so the chain can
                # start almost immediately
                if sname == "a" and pending_merge:
                    reload, combine, agfn = pending_merge.pop(0)
                    add_inject(inject, [reload], [6])
                    add_inject(inject, [combine], [10])
                    if agfn is not None:
                        add_inject(inject, [agfn], [14])
                is_last = (si == len(order) - 1 and sname == "b")
                yt = ya.tile([128, CACC, ES], FP32, tag="y")
                nc.vector.memset(yt[:], 0.0)
                yts[(hop_i, key)] = yt
                # self-scatters at computed safe points
                tgt = bnd[(hop_i, d, sname)]
                inj_pts, late_pts = scatter_points(s)
                for k, ci in inj_pts:
                    inject.setdefault(ci, []).append(
                        mk_scatter(yt, key, tgt, k))
                late = [mk_scatter(yt, key, tgt, k) for k, ci in late_pts]
                prop_stream(yt, s, key, *tabs[hop_i][d], estep=ES,
                            inject=inject)
                if is_last:
                    for fn in late:
                        fn()
                else:
                    pending = late
                # compute phases: emitted right after the A-stream that
                # carries their last data dependency (merge injections),
                # so they overlap the remaining gather streams
                if (hop_i, d, sname) == (2, "f", "a"):
                    early_phase()
                elif (hop_i, d, sname) == (2, "r", "a"):
                    blk2_phase()
            # after both windows: queue this hop's merge for the next
            # A-stream, and the AllGather for hop-1 merges
            if hop_i == 1:
                blk = 0 if d == "f" else 1
                scale = doe if d == "f" else die
                reload, combine = mk_merge(hop_i, d, blk, scale)
                pending_merge.append((reload, combine, mk_ag(d)))
            elif d == "f":
                reload, combine = mk_merge(2, "f", 2, None)
                pending_merge.append((reload, combine, None))

        # --- tail: Y2i merge + gates + head, pipelined per 896-node chunk --
        for gci in range(NSC):
            c0 = gci * CH
            c1 = min(c0 + CH, CACC)
            a = tl.tile([128, CH, F], FP32, tag="ta")
            b = tl.tile([128, CH, F], FP32, tag="tb")
            nc.sync.dma_start(a[:, :c1 - c0, :],
                               wrap(bnd[(2, "r", "a")], 0, F)[:, c0:c1, :])
            nc.sync.dma_start(b[:, :c1 - c0, :],
                               wrap(bnd[(2, "r", "b")], 0, F)[:, c0:c1, :])
            nc.vector.tensor_tensor(out=a[:], in0=a[:], in1=b[:],
                                    op=mybir.AluOpType.add)
            for j0 in range(0, c1 - c0, 4):
                jn = min(4, c1 - c0 - j0)
                ps = pt.tile([128, 128], FP32, tag="ps")
                nc.tensor.transpose(
                    out=ps[0:jn * F, :],
                    in_=a[:, j0:j0 + jn, :].rearrange("p a b -> p (a b)"),
                    identity=ident[:])
                for k in range(jn):
                    t = c0 + j0 + k
                    nc.vector.tensor_copy(
                        out=phi[0:F, t * 128:(t + 1) * 128],
                        in_=ps[k * F:(k + 1) * F, :])
            base = c0 * 128
            span = (c1 - c0) * 128
            half = span // 2
            for s0 in (base, base + half):
                w = half
                pg = pp.tile([GATES, NT], FP32, tag="pg")
                nc.tensor.matmul(out=pg[:, :w], lhsT=wa3[:, :],
                                 rhs=phi[0:F, s0:s0 + w],
                                 start=True, stop=True)
                gz = px.tile([FO, NT], FP32, tag="px")
                nc.vector.tensor_tensor(out=gz[:, :w], in0=pg[0:FO, :w],
                                        in1=gPZ[:, s0:s0 + w],
                                        op=mybir.AluOpType.add)
                gh = px.tile([FO, NT], FP32, tag="px")
                nc.vector.tensor_tensor(out=gh[:, :w], in0=pg[FO:GATES, :w],
                                        in1=gPH[:, s0:s0 + w],
                                        op=mybir.AluOpType.add)
                nc.scalar.activation(out=gz[:, :w], in_=gz[:, :w],
                                     func=AF.Sigmoid, bias=bS[:], scale=-1.0)
                nc.scalar.activation(out=gh[:, :w], in_=gh[:, :w],
                                     func=AF.Tanh, bias=bT[:], scale=1.0)
                nc.vector.tensor_tensor(out=gz[:, :w], in0=gz[:, :w],
                                        in1=gh[:, :w], op=mybir.AluOpType.mult)
                nc.vector.tensor_scalar_max(gz[:, :w], gz[:, :w], 0.0)
                po = ph.tile([PER, NT], FP32, tag="po")
                nc.tensor.matmul(out=po[:, :w], lhsT=wl[:],
                                 rhs=gz[:, :w], start=True, stop=True)
                ot = px.tile([PER, NT], FP32, tag="ot")
                nc.scalar.add(out=ot[:, :w], in_=po[:, :w], add=bl[:])
                nc.scalar.dma_start(out_d[:, s0:s0 + w], ot[:, :w])

    nc.compile()
    return nc


# ----------------------------------------------------------------------------
# Entry point
# ----------------------------------------------------------------------------

def _assemble(results, meta):
    N, P, NLOC = meta["N"], meta["P"], meta["NLOC"]
    out = np.empty((N, PER), np.float32)
    for p in range(P):
        out[p * NLOC:(p + 1) * NLOC] = results[p]["out"].T[:NLOC]
    return out


def kernel(x, edge_index, edge_weight, w_z, b_z, w_r, b_r, w_h, b_h,
           w_lin, b_lin, _trace=False, _window=WINDOW):
    in_maps, meta = preprocess(x, edge_index, edge_weight, w_z, b_z, w_r,
                               b_r, w_h, b_h, w_lin, b_lin, window=_window)
    nc = build_program(meta)
    res = bass_utils.run_bass_kernel_spmd(
        nc, in_maps, core_ids=list(range(meta["P"])), trace=_trace)
    out = _assemble(res.results, meta)
    if _trace:
        return out, res
    return out


# revision 32
# speedup vs baseline: 1.1239x; 1.1239x over previous
"""Trainium2 Bass kernel for DCRNN-Temporal (gnn_message_passing).

Contract: kernel(**inputs) takes FULL numpy inputs (x, edge_index,
edge_weight, w_z, b_z, w_r, b_r, w_h, b_h, w_lin, b_lin) and returns the
FULL [N, 12] output, running a Bass SPMD kernel on 8 NeuronCores.

Math (H0 = 0 simplifies the DCRNN cell):
  R is unused (H0*R = 0), both remaining DConvs share the same diffusion
  features. With per-edge weights 1/deg(src) folded into pre-scaled
  tables and the Chebyshev recurrence folded into effective weights:
    T1o = P_f x, T1i = P_r x, Y2o = P_f T1o, Y2i = P_r T1i
    G   = [x | T1o, T1i, Y2o, Y2i] @ Weff + beff    (z | h gates)
    H   = sigmoid(-Gz - bz) * tanh(Gh + bh)         (= (1-Z)*Htilde)
    out = relu(H) @ w_lin + b_lin

Sharding: nodes partitioned by destination across 8 cores.  Each
propagate = dma_gather row gathers (256B rows) + DVE adds over
degree-sorted ELL rounds (scatter-free).  dma_gather indices are int16,
so every node table uses one global 6256-periodic layout (6250 node
rows + 6 zero rows per core; gid(v) = (v//6250)*6256 + v%6250 < 50048)
and each direction's edges split into two windowed streams:
  A: gid < 32768 (idx = gid),  B: idx = gid - 32768 (table AP offset).

Performance structure (v3): SWDGE descriptor generation is the
bottleneck (~7.5ns/idx per queue, 4 queues generating concurrently).
The 8 gather streams (2 hops x 2 dirs x 2 windows) issue back-to-back;
each stream's un-permute scatter_adds are issued in REVERSE column
order (low-degree accumulator columns receive their last ELL add early)
and injected into the NEXT stream's gather rotation, so the gpsimd
in-order queue never stalls on yt-finalization.  Merge reloads ride the
scalar queue; ix index loads own the sync queue.  The AllGathers are
injected mid-stream so their transfer hides under SWDGE work.  The gate
computation is split: x/T1 parts during hop-2 fwd, Y2o part during
hop-2 rev, and only the Y2i part in the tail, pipelined per 896-node
chunk behind the final scatters.
"""

import os
import sys

for _p in ("/opt/trn_rl_repo", "/root/.axon_site/_ro/trn_rl_repo"):
    if os.path.isdir(_p) and _p not in sys.path:
        sys.path.insert(0, _p)
        break

import numpy as np

import concourse.bass as bass
import concourse.tile as tile
from concourse import bacc, mybir
from concourse import bass_utils
from concourse.masks import make_identity

F = 32          # node feature dim
FO = 64         # out channels per gate
GATES = 2 * FO
PER = 12        # head output dim
NCORES = 8
ES = 64         # table row length (f32) = 256B, required by dma_gather
CH = 7          # SWDGE chunk width in 128-cols (<=896 descriptors/call)
NT = 512        # node tile (free dim) for gate matmuls
WINDOW = 32768  # int16 idx window
GAPZ = 6        # zero rows appended per core in the global table layout
NQ = 4          # SWDGE queues (ucode max); round-robin for parallel gen

FP32 = mybir.dt.float32
I16 = mybir.dt.int16


def _ceil(a, b):
    return -(-a // b)


def _wrap16(idx_seq):
    """[n] -> [128, n//16] int16, wrapped in 16 partitions, replicated 8x."""
    w = idx_seq.reshape(-1, 16).T.astype(np.int16)
    return np.ascontiguousarray(np.tile(w, (8, 1)))


# ----------------------------------------------------------------------------
# Host-side graph preprocessing
# ----------------------------------------------------------------------------

def _build_streams(dst, src_gid, N, NLOC, P, W):
    """ELL-round structures for one directed edge set, split into
    idx-window streams A (gid < W) and B (gid >= W)."""
    core = dst // NLOC
    dl = dst - core * NLOC
    streams = []
    for s, mask in (("A", src_gid < W), ("B", src_gid >= W)):
        percore = []
        for p in range(P):
            sel = np.nonzero((core == p) & mask)[0]
            cnt = np.bincount(dl[sel], minlength=NLOC)
            perm = np.argsort(-cnt, kind="stable").astype(np.int32)
            rank = np.empty(NLOC, np.int32)
            rank[perm] = np.arange(NLOC, dtype=np.int32)
            order = np.argsort(dl[sel], kind="stable")
            es = sel[order]
            ptr = np.zeros(NLOC + 1, np.int64)
            np.cumsum(cnt, out=ptr[1:])
            j_arr = np.arange(len(es), dtype=np.int64) - ptr[dl[es]]
            percore.append(dict(es=es, j=j_arr, r=rank[dl[es]].astype(np.int64),
                                cnt=cnt, perm=perm))
        maxdeg = int(max(pc["cnt"].max(initial=0) for pc in percore))
        NJ = []
        for j in range(maxdeg):
            nj = max(int((pc["cnt"] > j).sum()) for pc in percore)
            NJ.append(_ceil(nj, 128) * 128)
        off = np.zeros(len(NJ) + 1, np.int64)
        np.cumsum(NJ, out=off[1:])
        EP = int(off[-1])
        for pc in percore:
            pc["slot"] = off[pc["j"]] + pc["r"]
        streams.append(dict(name=s, percore=percore, NJ=NJ, off=off, EP=EP))
    return streams


def preprocess(x, edge_index, edge_weight, w_z, b_z, w_r, b_r, w_h, b_h,
               w_lin, b_lin, P=NCORES, window=WINDOW):
    N, Fx = x.shape
    assert Fx == F
    assert N % P == 0
    NLOC = N // P
    NLOCP = _ceil(NLOC, 128) * 128
    CACC = NLOCP // 128
    NL6 = NLOC + GAPZ
    NTAB = P * NL6
    W = min(window, NTAB)
    assert NTAB - W < 32768 and NLOC < W

    row = np.asarray(edge_index[0], dtype=np.int64)
    col = np.asarray(edge_index[1], dtype=np.int64)
    ew = np.asarray(edge_weight, dtype=np.float64)
    deg_out = np.bincount(row, weights=ew, minlength=N)
    deg_in = np.bincount(col, weights=ew, minlength=N)
    with np.errstate(divide="ignore"):
        doi = np.where(deg_out > 0, 1.0 / deg_out, 0.0).astype(np.float32)
        dii = np.where(deg_in > 0, 1.0 / deg_in, 0.0).astype(np.float32)

    gid = (np.arange(N) // NLOC) * NL6 + (np.arange(N) % NLOC)
    xf = np.asarray(x, np.float32)

    def table(scaled):
        t = np.zeros((NTAB, ES), np.float32)
        t[gid, :F] = scaled
        return t

    XF = table(xf * doi[:, None])
    XR = table(xf * dii[:, None])

    # pad slot targets: a zero row inside each window
    apad = NLOC  # core-0 zero row, < W
    zq = _ceil(W - NLOC, NL6)  # first core whose zero row is >= W
    bpad = zq * NL6 + NLOC - W
    assert 0 <= bpad < NTAB - W or W == NTAB

    gsrcF = gid[row]
    gsrcR = gid[col]
    stF = _build_streams(col, gsrcF, N, NLOC, P, W)
    stR = _build_streams(row, gsrcR, N, NLOC, P, W)

    # effective gate weights (K = 3); WA rows = [T1o, T1i, Y2o, Y2i] so the
    # T1 half [0:2F] can matmul early (during hop 2) with contiguous lhsT
    assert w_z.shape[1] == 3

    def gate_w(w):
        w = np.asarray(w, np.float32)
        wx = w[0, 0, :F] + w[1, 0, :F] - w[0, 2, :F] - w[1, 2, :F]
        wa = np.concatenate(
            [w[0, 1, :F], w[1, 1, :F], 2.0 * w[0, 2, :F], 2.0 * w[1, 2, :F]], 0)
        return wx, wa

    wxz, waz = gate_w(w_z)
    wxh, wah = gate_w(w_h)
    WA = np.ascontiguousarray(np.concatenate([waz, wah], axis=1))
    WX = np.ascontiguousarray(np.concatenate([wxz, wxh], axis=1))
    biasS = np.ascontiguousarray(-np.asarray(b_z, np.float32)[:, None])
    biasT = np.ascontiguousarray(np.asarray(b_h, np.float32)[:, None])
    WL = np.asarray(w_lin, np.float32)
    BL = np.ascontiguousarray(np.asarray(b_lin, np.float32)[:, None])

    def expand(vec):  # [NLOCP] -> [128, CACC, F] accumulator-layout expand
        return np.ascontiguousarray(
            np.repeat(vec.reshape(CACC, 128).T, F, axis=1)
        ).reshape(128, CACC, F)

    in_maps = []
    for p in range(P):
        m = {"xf": XF, "xr": XR, "wa": WA, "wx": WX, "biass": biasS,
             "biast": biasT, "wl": WL, "bl": BL}
        for dname, st, gsrc, srcv in (("f", stF, gsrcF, row),
                                      ("r", stR, gsrcR, col)):
            for s in st:
                pc = s["percore"][p]
                if s["EP"] == 0:
                    continue
                base, pad = (0, apad) if s["name"] == "A" else (W, bpad)
                ivals = np.full(s["EP"], pad, np.int64)
                ivals[pc["slot"]] = gsrc[pc["es"]] - base
                m[f"ix{dname}{s['name'].lower()}"] = _wrap16(ivals)
                sidx = np.full(NLOCP, NLOC, np.int64)
                sidx[:NLOC] = pc["perm"]
                m[f"sx{dname}{s['name'].lower()}"] = _wrap16(sidx)
        sl = slice(p * NLOC, (p + 1) * NLOC)
        xp = np.zeros((NLOCP, F), np.float32)
        xp[:NLOC] = xf[sl]
        m["xpt"] = np.ascontiguousarray(xp.T)
        tmp = np.zeros(NLOCP, np.float32)
        tmp[:NLOC] = doi[sl]
        m["doe"] = expand(tmp)
        tmp = np.zeros(NLOCP, np.float32)
        tmp[:NLOC] = dii[sl]
        m["die"] = expand(tmp)
        in_maps.append(m)

    meta = dict(N=N, P=P, NLOC=NLOC, NLOCP=NLOCP, CACC=CACC, NL6=NL6,
                NTAB=NTAB, W=W,
                streams={"f": [dict(name=s["name"], NJ=s["NJ"], off=s["off"],
                                    EP=s["EP"]) for s in stF],
                         "r": [dict(name=s["name"], NJ=s["NJ"], off=s["off"],
                                    EP=s["EP"]) for s in stR]})
    return in_maps, meta


# ----------------------------------------------------------------------------
# Device program
# ----------------------------------------------------------------------------

def build_program(meta, debug=False):
    from contextlib import ExitStack

    N, P = meta["N"], meta["P"]
    NLOC, NLOCP, CACC = meta["NLOC"], meta["NLOCP"], meta["CACC"]
    NL6, NTAB, W = meta["NL6"], meta["NTAB"], meta["W"]
    streams = meta["streams"]
    NSC = _ceil(CACC, CH)  # scatter chunks per stream (7)

    nc = bacc.Bacc("TRN2", target_bir_lowering=False, debug=False,
                   num_devices=P, num_swdge_queues=NQ)

    def din(name, shape, dt=FP32):
        return nc.dram_tensor(name, list(shape), dt, kind="ExternalInput").ap()

    def dout(name, shape, dt=FP32):
        return nc.dram_tensor(name, list(shape), dt, kind="ExternalOutput").ap()

    xf_d = din("xf", (NTAB, ES))
    xr_d = din("xr", (NTAB, ES))
    ix_d, sx_d = {}, {}
    for d in ("f", "r"):
        for s in streams[d]:
            if s["EP"] == 0:
                continue
            k = d + s["name"].lower()
            ix_d[k] = din("ix" + k, (128, s["EP"] // 16), I16)
            sx_d[k] = din("sx" + k, (128, NLOCP // 16), I16)
    xpt_d = din("xpt", (F, NLOCP))
    doe_d = din("doe", (128, CACC, F))
    die_d = din("die", (128, CACC, F))
    wa_d = din("wa", (4 * F, GATES))
    wx_d = din("wx", (F, GATES))
    biass_d = din("biass", (FO, 1))
    biast_d = din("biast", (FO, 1))
    wl_d = din("wl", (FO, PER))
    bl_d = din("bl", (PER, 1))
    out_d = dout("out", (PER, NLOCP))

    EPmax = max(s["EP"] for d in ("f", "r") for s in streams[d])
    qctr = [0]

    def nextq():
        q = qctr[0] % NQ
        qctr[0] += 1
        return q

    with tile.TileContext(nc) as tc, ExitStack() as ctx:
        sb = ctx.enter_context(tc.tile_pool(name="sb", bufs=1))
        ya = ctx.enter_context(tc.tile_pool(name="ya", bufs=3))
        gp = ctx.enter_context(tc.tile_pool(name="gp", bufs=9))
        ixa = ctx.enter_context(tc.tile_pool(name="ixa", bufs=1))
        ixb = ctx.enter_context(tc.tile_pool(name="ixb", bufs=1))
        big = ctx.enter_context(tc.tile_pool(name="big", bufs=1))
        mg = ctx.enter_context(tc.tile_pool(name="mg", bufs=1))
        tl = ctx.enter_context(tc.tile_pool(name="tl", bufs=4))
        px = ctx.enter_context(tc.tile_pool(name="px", bufs=2))
        pp = ctx.enter_context(tc.tile_pool(name="pp", bufs=2, space="PSUM"))
        pt = ctx.enter_context(tc.tile_pool(name="pt", bufs=2, space="PSUM"))
        ph = ctx.enter_context(tc.tile_pool(name="ph", bufs=2, space="PSUM"))
        dr = ctx.enter_context(tc.tile_pool(name="dr", bufs=1, space="DRAM"))

        phi = sb.tile([128, NLOCP], FP32, tag="phi")
        gPZ = sb.tile([FO, NLOCP], FP32, tag="gPZ")
        gPH = sb.tile([FO, NLOCP], FP32, tag="gPH")
        doe = sb.tile([128, CACC, F], FP32, tag="doe")
        die = sb.tile([128, CACC, F], FP32, tag="die")
        sxt = {}
        for d in ("f", "r"):
            for s in streams[d]:
                if s["EP"] == 0:
                    continue
                k = d + s["name"].lower()
                sxt[k] = sb.tile([128, NLOCP // 16], I16, tag="sx" + k,
                                 name="sx" + k)
        wa = sb.tile([4 * F, GATES], FP32, tag="wa")
        wa3 = sb.tile([F, GATES], FP32, tag="wa3")  # Y2i block at base 0
        wx = sb.tile([F, GATES], FP32, tag="wx")
        bS = sb.tile([FO, 1], FP32, tag="bS")
        bT = sb.tile([FO, 1], FP32, tag="bT")
        wl = sb.tile([FO, PER], FP32, tag="wl")
        bl = sb.tile([PER, 1], FP32, tag="bl")
        ident = sb.tile([128, 128], FP32, tag="ident")

        # yall holds natural-order merged T1 blocks (T1o | T1i); Y2o gets its
        # own tile; the Y2i block goes straight to phi in the tail.
        yall = big.tile([128, CACC, 2 * F], FP32, tag="big")
        y2o = sb.tile([128, CACC, F], FP32, tag="y2o")

        NBNC = _ceil(max(NLOCP, NL6), 128) * 128
        # per-stream scatter targets (A/B separated so their scatter_adds
        # run concurrently on different queues; merged on reload)
        bnd = {}
        for hop_i in (1, 2):
            for d in ("f", "r"):
                for s in ("a", "b"):
                    nm = f"bnd{hop_i}{d}{s}"
                    bnd[(hop_i, d, s)] = dr.tile([NBNC, ES], FP32,
                                                 tag=nm, name=nm)
        bounS = {d: dr.tile([NBNC, ES], FP32, tag="bounS" + d,
                            name="bounS" + d) for d in ("f", "r")}
        # Shared HBM output lets the AllGather write in place (each core
        # deposits its slab) instead of the slow HBM-HBM collective path.
        ag = {d: nc.dram_tensor("agsh" + d, [NTAB, ES], FP32,
                                addr_space="Shared").ap()
              for d in ("f", "r")}

        def wrap(dram_tile, c0, cn):
            apv = dram_tile[:].rearrange("(c p) f -> p c f", p=128)
            return apv[:, :, c0:c0 + cn]

        # --- loads: ix of the first stream rides the sync queue alone;
        # everything else goes on scalar so the first gather starts asap.
        for k, t in sxt.items():
            nc.scalar.dma_start(t[:], sx_d[k])
        nc.scalar.dma_start(doe[:], doe_d)
        nc.scalar.dma_start(die[:], die_d)
        nc.scalar.dma_start(wa[:], wa_d)
        nc.scalar.dma_start(wa3[:], wa_d[3 * F:4 * F, :])
        nc.scalar.dma_start(wx[:], wx_d)
        nc.scalar.dma_start(bS[:], biass_d)
        nc.scalar.dma_start(bT[:], biast_d)
        nc.scalar.dma_start(wl[:], wl_d)
        nc.scalar.dma_start(bl[:], bl_d)
        make_identity(nc, ident[:])

        # --- prezero DRAM scatter targets (scalar queue, early) ------------
        z0 = ya.tile([128, CACC, ES], FP32, tag="y", name="z0")
        nc.vector.memset(z0[:], 0.0)
        for t in bnd.values():
            nc.scalar.dma_start(wrap(t, 0, ES)[:, :CACC, :], z0[:])

        # --- stream machinery ----------------------------------------------
        def prop_stream(yt, sdesc, ixkey, tabA, tabB, estep, inject):
            """Gather + ELL-round accumulate for one windowed stream.
            `inject` maps chunk index -> list of closures to emit there
            (scatters of previous streams, merge reloads, AllGathers)."""
            EP = sdesc["EP"]
            EPc = EP // 128
            roff = [int(o) // 128 for o in sdesc["off"]]
            nrounds = len(sdesc["NJ"])
            table_ap = tabA if sdesc["name"] == "A" else tabB
            pool = ixa if sdesc["name"] == "A" else ixb
            ixt = pool.tile([128, EPmax // 16 if sdesc["name"] == "A"
                             else EP // 16], I16, tag="ix")
            nc.sync.dma_start(ixt[:, :EP // 16], ix_d[ixkey])
            nch = _ceil(EPc, CH)
            for ci, c0 in enumerate(range(0, EPc, CH)):
                for fn in inject.pop(ci, ()):
                    fn()
                c1 = min(c0 + CH, EPc)
                g = gp.tile([128, CH, ES], FP32, tag="g")
                nc.gpsimd.dma_gather(
                    out_ap=g[:, :c1 - c0, :],
                    in_ap=table_ap,
                    idxs_ap=ixt[:, c0 * 8:c1 * 8],
                    num_idxs=(c1 - c0) * 128,
                    num_idxs_reg=(c1 - c0) * 128,
                    elem_size=ES,
                    elem_step=estep,
                    queue_num=nextq(),
                )
                for j in range(nrounds):
                    s = max(roff[j], c0)
                    e = min(roff[j + 1], c1)
                    if s >= e:
                        continue
                    ys = s - roff[j]
                    nc.vector.tensor_tensor(
                        out=yt[:, ys:ys + e - s, 0:F],
                        in0=yt[:, ys:ys + e - s, 0:F],
                        in1=g[:, s - c0:e - c0, 0:F],
                        op=mybir.AluOpType.add,
                    )
            # flush any injections past the end
            for ci in sorted(inject):
                for fn in inject.pop(ci):
                    fn()

        def mk_scatter(yt, key, tgt, k):
            """Scatter chunk k (columns [7k, 7k+7)) of yt into tgt."""
            def fn():
                c0 = k * CH
                c1 = min(c0 + CH, CACC)
                nc.gpsimd.dma_scatter_add(
                    out_ap=tgt[:],
                    in_ap=yt[:, c0:c1, :],
                    idxs_ap=sxt[key][:, c0 * 8:c1 * 8],
                    num_idxs=(c1 - c0) * 128,
                    num_idxs_reg=(c1 - c0) * 128,
                    elem_size=ES,
                    queue_num=nextq(),
                )
            return fn

        def add_inject(inject, closures, cis):
            for fn, ci in zip(closures, cis):
                inject.setdefault(ci, []).append(fn)

        def mk_merge(hop_i, d, blk, scale_t):
            """Reload A+B scatter buffers (scalar queue), merge into yall
            block `blk` (or y2o); for hop 1 also write the scaled AllGather
            input.  Returns (reload_closure, combine_closure)."""
            dst = (y2o[:, :, :] if blk == 2
                   else yall[:, :, blk * F:(blk + 1) * F])
            tmp = mg.tile([128, CACC, F], FP32, tag="mgl", name=f"mg{hop_i}{d}")

            def reload():
                nc.sync.dma_start(
                    dst, wrap(bnd[(hop_i, d, "a")], 0, F)[:, :CACC, :])
                nc.sync.dma_start(
                    tmp[:], wrap(bnd[(hop_i, d, "b")], 0, F)[:, :CACC, :])

            def combine():
                nc.vector.tensor_tensor(out=dst, in0=dst, in1=tmp[:],
                                        op=mybir.AluOpType.add)
                if scale_t is None:
                    return
                ts = mg.tile([128, CACC, F], FP32, tag="mgs")
                nc.vector.tensor_tensor(out=ts[:], in0=dst, in1=scale_t[:],
                                        op=mybir.AluOpType.mult)
                nc.sync.dma_start(wrap(bounS[d], 0, F)[:, :CACC, :], ts[:])

            return reload, combine

        def mk_ag(d):
            def fn():
                nc.gpsimd.collective_compute(
                    "AllGather", mybir.AluOpType.bypass,
                    replica_groups=[list(range(P))],
                    ins=[bounS[d][0:NL6, :].opt()],
                    outs=[ag[d][0:NTAB, :].opt()],
                )
            return fn

        # --- emission ------------------------------------------------------
        sdesc = {}
        for d in ("f", "r"):
            for s in streams[d]:
                sdesc[d + s["name"].lower()] = s

        tabs = {
            1: {"f": (xf_d[0:W, :], xf_d[W:NTAB, :]),
                "r": (xr_d[0:W, :], xr_d[W:NTAB, :])},
            2: {"f": (ag["f"][0:W, :], ag["f"][W:NTAB, :]),
                "r": (ag["r"][0:W, :], ag["r"][W:NTAB, :])},
        }

        LATE_SPREAD = [1, 3, 5, 7, 9, 11, 13]

        def scatter_points(s):
            """Safe self-injection chunk index for each scatter chunk k:
            columns [7k, 7k+7) receive their last ELL add once the final
            round with NJ > 896k has been gathered (+2 chunks of drain
            margin).  Chunks whose point falls past the stream end are
            returned separately for injection into the next stream."""
            EPc = s["EP"] // 128
            nch = _ceil(EPc, CH)
            inj, late = [], []
            for k in reversed(range(NSC)):
                lim = 128 * CH * k
                jlast = max(j for j, nj in enumerate(s["NJ"]) if nj > lim)
                ci = _ceil(int(s["off"][jlast + 1]), 128 * CH) + 2
                (inj if ci < nch else late).append((k, ci))
            return inj, late

        order = [(1, "f"), (1, "r"), (2, "f"), (2, "r")]
        yts = {}
        pending = []          # leftover scatter closures for the next stream
        pending_merge = []    # (reload, combine, agfn or None)

        AF = mybir.ActivationFunctionType

        def early_phase():
            # T1 transposes (2 cols per 128x128 PE transpose) + x/T1 partial
            # gates (runs during hop 2 fwd); psum->SBUF copies ride the
            # scalar engine so the DVE queue stays clear for ELL adds
            for t0 in range(0, CACC, 2):
                tn = min(2, CACC - t0)
                ps = pt.tile([128, 128], FP32, tag="ps")
                nc.tensor.transpose(
                    out=ps[0:tn * 2 * F, :],
                    in_=yall[:, t0:t0 + tn, :].rearrange("p a b -> p (a b)"),
                    identity=ident[:])
                for k in range(tn):
                    t = t0 + k
                    nc.scalar.copy(phi[0:FO, t * 128:(t + 1) * 128],
                                   ps[k * 2 * F:(k + 1) * 2 * F, :])
            for n0 in range(0, NLOCP, NT):
                n1 = min(n0 + NT, NLOCP)
                w = n1 - n0
                pxt = px.tile([F, NT], FP32, tag="pxt")
                nc.sync.dma_start(pxt[:, :w], xpt_d[:, n0:n1])
                pg = pp.tile([GATES, NT], FP32, tag="pg")
                nc.tensor.matmul(out=pg[:, :w], lhsT=wa[0:2 * F, :],
                                 rhs=phi[0:FO, n0:n1], start=True, stop=False)
                nc.tensor.matmul(out=pg[:, :w], lhsT=wx[:, :],
                                 rhs=pxt[:, :w], start=False, stop=True)
                nc.scalar.copy(gPZ[:, n0:n1], pg[0:FO, :w])
                nc.scalar.copy(gPH[:, n0:n1], pg[FO:GATES, :w])

        def blk2_phase():
            # Y2o transposes + partial gates accumulated into gPZ/gPH
            # (runs during hop 2 rev)
            for t0 in range(0, CACC, 4):
                tn = min(4, CACC - t0)
                ps = pt.tile([128, 128], FP32, tag="ps")
                nc.tensor.transpose(
                    out=ps[0:tn * F, :],
                    in_=y2o[:, t0:t0 + tn, :].rearrange("p a b -> p (a b)"),
                    identity=ident[:])
                for k in range(tn):
                    t = t0 + k
                    nc.scalar.copy(phi[FO:FO + F, t * 128:(t + 1) * 128],
                                   ps[k * F:(k + 1) * F, :])
            for n0 in range(0, NLOCP, NT):
                n1 = min(n0 + NT, NLOCP)
                w = n1 - n0
                pg = pp.tile([GATES, NT], FP32, tag="pg")
                nc.tensor.matmul(out=pg[:, :w], lhsT=wa[2 * F:3 * F, :],
                                 rhs=phi[FO:FO + F, n0:n1],
                                 start=True, stop=True)
                nc.vector.tensor_tensor(out=gPZ[:, n0:n1], in0=gPZ[:, n0:n1],
                                        in1=pg[0:FO, :w], op=mybir.AluOpType.add)
                nc.vector.tensor_tensor(out=gPH[:, n0:n1], in0=gPH[:, n0:n1],
                                        in1=pg[FO:GATES, :w],
                                        op=mybir.AluOpType.add)

        for si, (hop_i, d) in enumerate(order):
            for sname in ("a", "b"):
                key = d + sname
                s = sdesc[key]
                inject = {}
                # leftover scatter chunks of the previous stream
                if pending:
                    add_inject(inject, pending, LATE_SPREAD)
                    pending = []
                # merge + AllGather of the previous hop (A-streams): the
                # previous streams fully self-scattered, so the chain can
                # start almost immediately
                if sname == "a" and pending_merge:
                    reload, combine, agfn = pending_merge.pop(0)
                    add_inject(inject, [reload], [6])
                    add_inject(inject, [combine], [10])
                    if agfn is not None:
                        add_inject(inject, [agfn], [14])
                is_last = (si == len(order) - 1 and sname == "b")
                yt = ya.tile([128, CACC, ES], FP32, tag="y")
                nc.vector.memset(yt[:], 0.0)
                yts[(hop_i, key)] = yt
                # self-scatters at computed safe points
                tgt = bnd[(hop_i, d, sname)]
                inj_pts, late_pts = scatter_points(s)
                for k, ci in inj_pts:
                    inject.setdefault(ci, []).append(
                        mk_scatter(yt, key, tgt, k))
                late = [mk_scatter(yt, key, tgt, k) for k, ci in late_pts]
                prop_stream(yt, s, key, *tabs[hop_i][d], estep=ES,
                            inject=inject)
                if is_last:
                    for fn in late:
                        fn()
                else:
                    pending = late
                # compute phases: emitted right after the A-stream that
                # carries their last data dependency (merge injections),
                # so they overlap the remaining gather streams
                if (hop_i, d, sname) == (2, "f", "a"):
                    early_phase()
                elif (hop_i, d, sname) == (2, "r", "a"):
                    blk2_phase()
            # after both windows: queue this hop's merge for the next
            # A-stream, and the AllGather for hop-1 merges
            if hop_i == 1:
                blk = 0 if d == "f" else 1
                scale = doe if d == "f" else die
                reload, combine = mk_merge(hop_i, d, blk, scale)
                pending_merge.append((reload, combine, mk_ag(d)))
            elif d == "f":
                reload, combine = mk_merge(2, "f", 2, None)
                pending_merge.append((reload, combine, None))

        # --- tail: Y2i merge + gates + head, pipelined per 896-node chunk --
        for gci in range(NSC):
            c0 = gci * CH
            c1 = min(c0 + CH, CACC)
            a = tl.tile([128, CH, F], FP32, tag="ta")
            b = tl.tile([128, CH, F], FP32, tag="tb")
            nc.sync.dma_start(a[:, :c1 - c0, :],
                               wrap(bnd[(2, "r", "a")], 0, F)[:, c0:c1, :])
            nc.sync.dma_start(b[:, :c1 - c0, :],
                               wrap(bnd[(2, "r", "b")], 0, F)[:, c0:c1, :])
            nc.vector.tensor_tensor(out=a[:], in0=a[:], in1=b[:],
                                    op=mybir.AluOpType.add)
            for j0 in range(0, c1 - c0, 4):
                jn = min(4, c1 - c0 - j0)
                ps = pt.tile([128, 128], FP32, tag="ps")
                nc.tensor.transpose(
                    out=ps[0:jn * F, :],
                    in_=a[:, j0:j0 + jn, :].rearrange("p a b -> p (a b)"),
                    identity=ident[:])
                for k in range(jn):
                    t = c0 + j0 + k
                    nc.vector.tensor_copy(
                        out=phi[0:F, t * 128:(t + 1) * 128],
                        in_=ps[k * F:(k + 1) * F, :])
            base = c0 * 128
            span = (c1 - c0) * 128
            half = span // 2
            for s0 in (base, base + half):
                w = half
                pg = pp.tile([GATES, NT], FP32, tag="pg")
                nc.tensor.matmul(out=pg[:, :w], lhsT=wa3[:, :],
                                 rhs=phi[0:F, s0:s0 + w],
                                 start=True, stop=True)
                gz = px.tile([FO, NT], FP32, tag="px")
                nc.vector.tensor_tensor(out=gz[:, :w], in0=pg[0:FO, :w],
                                        in1=gPZ[:, s0:s0 + w],
                                        op=mybir.AluOpType.add)
                gh = px.tile([FO, NT], FP32, tag="px")
                nc.vector.tensor_tensor(out=gh[:, :w], in0=pg[FO:GATES, :w],
                                        in1=gPH[:, s0:s0 + w],
                                        op=mybir.AluOpType.add)
                nc.scalar.activation(out=gz[:, :w], in_=gz[:, :w],
                                     func=AF.Sigmoid, bias=bS[:], scale=-1.0)
                nc.scalar.activation(out=gh[:, :w], in_=gh[:, :w],
                                     func=AF.Tanh, bias=bT[:], scale=1.0)
                nc.vector.tensor_tensor(out=gz[:, :w], in0=gz[:, :w],
                                        in1=gh[:, :w], op=mybir.AluOpType.mult)
                nc.vector.tensor_scalar_max(gz[:, :w], gz[:, :w], 0.0)
                po = ph.tile([PER, NT], FP32, tag="po")
                nc.tensor.matmul(out=po[:, :w], lhsT=wl[:],
                                 rhs=gz[:, :w], start=True, stop=True)
                ot = px.tile([PER, NT], FP32, tag="ot")
                nc.scalar.add(out=ot[:, :w], in_=po[:, :w], add=bl[:])
                nc.scalar.dma_start(out_d[:, s0:s0 + w], ot[:, :w])

    nc.compile()
    return nc


# ----------------------------------------------------------------------------
# Entry point
# ----------------------------------------------------------------------------

def _assemble(results, meta):
    N, P, NLOC = meta["N"], meta["P"], meta["NLOC"]
    out = np.empty((N, PER), np.float32)
    for p in range(P):
        out[p * NLOC:(p + 1) * NLOC] = results[p]["out"].T[:NLOC]
    return out


def kernel(x, edge_index, edge_weight, w_z, b_z, w_r, b_r, w_h, b_h,
           w_lin, b_lin, _trace=False, _window=WINDOW):
    in_maps, meta = preprocess(x, edge_index, edge_weight, w_z, b_z, w_r,
                               b_r, w_h, b_h, w_lin, b_lin, window=_window)
    nc = build_program(meta)
    res = bass_utils.run_bass_kernel_spmd(
        nc, in_maps, core_ids=list(range(meta["P"])), trace=_trace)
    out = _assemble(res.results, meta)
    if _trace:
        return out, res
    return out
